# revision 7
# baseline (speedup 1.0000x reference)
"""GCN layer kernel for Trainium2 (8 NeuronCores) — full computation on device.

Reference: Z = X @ W; agg = segment_sum(Z[edge_src] * edge_vals, edge_dst);
out = relu(agg).

Strategy: use linearity to reorder — relu((A_hat X) W) instead of
relu(A_hat (X W)).  Aggregating X FIRST means no core ever computes the full
50k-row projection (the baseline replicated a 26-GFLOP matmul on every core
and round-tripped a 51 MB Z through DRAM).  Each core owns 1/8 of the dst
nodes and:
  per 128-dst tile: Q7 dma_gather fetches the X rows of its source nodes
  (bf16, two tables split at row 25088 since gather indices are int16);
  a per-block selection matrix S[e,d] = (iota==slot)*w (built on DVE) folds
  the weighted segment-sum into PSUM-accumulated matmuls aggX[d,f] += S^T G;
  PE transposes aggX via identity-matmul; then a 4-step matmul with the
  replicated W [512,512] projects, and ReLU goes out.

Host does graph partitioning: dst nodes are packed 2D-greedily into
128-row tiles balancing lo/hi edge counts (so every tile needs exactly
NBL+NBH gather blocks with ~1% padding), plus the final unpermute.
"""

import numpy as np

N_NODES = 50000
M_IN = 512
H_OUT = 512
N_CORES = 8
P = 128
KC = M_IN // P                    # 4 contraction chunks for the projection
SPLIT = 25088                     # X table split (int16 gather index range)
XPAD = 2 * SPLIT                  # 50176 padded X rows
NT_PC = 50                        # dst tiles per core
NTG = N_CORES * NT_PC             # 400 global tiles
ROWS_PC = NT_PC * P               # 6400 out rows per core

_compiled = {}


def _build_nc(NBL, NBH, reps=1, nq=4):
    from contextlib import ExitStack
    from concourse import bacc, mybir
    from concourse import tile

    f32 = mybir.dt.float32
    bf16 = mybir.dt.bfloat16
    i16 = mybir.dt.int16
    NB = NBL + NBH

    # Each SWDGE queue_num runs descriptor generation on its own Q7 core
    # pair (dma_gather.cpp: cpu_id / 2 == queue_num), so rotating gathers
    # across 4 queues parallelizes the descgen that bounds this kernel.
    nc = bacc.Bacc(None, debug=False, num_swdge_queues=nq)

    xr = nc.declare_dram_parameter("xr", [XPAD, M_IN], bf16, isOutput=False)
    w = nc.declare_dram_parameter("w", [M_IN, H_OUT], bf16, isOutput=False)
    eidxlo = nc.declare_dram_parameter(
        "eidxlo", [ROWS_PC, NBL * 8], i16, isOutput=False
    )
    eidxhi = nc.declare_dram_parameter(
        "eidxhi", [ROWS_PC, NBH * 8], i16, isOutput=False
    )
    esw = nc.declare_dram_parameter("esw", [ROWS_PC, 2, NB], f32, isOutput=False)
    out = nc.declare_dram_parameter("out", [ROWS_PC, H_OUT], f32, isOutput=True)

    with tile.TileContext(nc) as tc:
        with ExitStack() as ctx:
            wpool = ctx.enter_context(tc.tile_pool(name="wpool", bufs=1))
            apool = ctx.enter_context(tc.tile_pool(name="apool", bufs=6))
            gpool = ctx.enter_context(tc.tile_pool(name="gpool", bufs=6))
            spool = ctx.enter_context(tc.tile_pool(name="spool", bufs=4))
            xpool = ctx.enter_context(tc.tile_pool(name="xpool", bufs=3))
            tpool = ctx.enter_context(tc.tile_pool(name="tpool", bufs=3))
            opool = ctx.enter_context(tc.tile_pool(name="opool", bufs=3))
            spp = ctx.enter_context(tc.tile_pool(name="spp", bufs=2, space="PSUM"))
            tpp = ctx.enter_context(tc.tile_pool(name="tpp", bufs=2, space="PSUM"))
            opp = ctx.enter_context(tc.tile_pool(name="opp", bufs=2, space="PSUM"))

            # reps>1 re-emits the whole computation (timing NEFFs only;
            # kernel() always uses reps=1)
            for _rep in range(reps):
                # ---- constants ----
                wsb = wpool.tile([P, KC, H_OUT], bf16, tag="wsb")
                for k in range(KC):
                    nc.sync.dma_start(wsb[:, k, :], w[k * P : (k + 1) * P, :])
                iota = wpool.tile([P, P], f32, tag="iota")
                nc.gpsimd.iota(
                    iota[:], pattern=[[1, P]], base=0, channel_multiplier=0,
                    allow_small_or_imprecise_dtypes=True,
                )
                piota = wpool.tile([P, 1], f32, tag="piota")
                nc.gpsimd.iota(
                    piota[:], pattern=[[0, 1]], base=0, channel_multiplier=1,
                    allow_small_or_imprecise_dtypes=True,
                )
                ident = wpool.tile([P, P], bf16, tag="ident")
                nc.vector.tensor_scalar(
                    ident[:], iota[:], piota[:, 0:1], None,
                    mybir.AluOpType.is_equal,
                )

                # ---- per dst tile: gather + select-matmul + transpose+proj ----
                for t in range(NT_PC):
                    r0 = t * P
                    idxlo_sb = apool.tile([P, NBL * 8], i16, tag="idxlo")
                    nc.sync.dma_start(idxlo_sb[:], eidxlo[r0 : r0 + P, :])
                    idxhi_sb = apool.tile([P, NBH * 8], i16, tag="idxhi")
                    nc.sync.dma_start(idxhi_sb[:], eidxhi[r0 : r0 + P, :])
                    sw_sb = apool.tile([P, 2, NB], f32, tag="sw")
                    nc.sync.dma_start(sw_sb[:], esw[r0 : r0 + P, :, :])

                    g = gpool.tile([P, NB, H_OUT], bf16)
                    # dma_gather breaks on HW above 1024 indices per call (the
                    # 1024-descriptor SWDGE ring) -> chunk into <=8-block calls
                    CH = 8
                    for nb, lohi, isb, off in (
                        (NBL, 0, idxlo_sb, 0),
                        (NBH, 1, idxhi_sb, NBL),
                    ):
                        for c0 in range(0, nb, CH):
                            cn = min(CH, nb - c0)
                            nc.gpsimd.dma_gather(
                                g[:, off + c0 : off + c0 + cn, :],
                                xr[lohi * SPLIT : (lohi + 1) * SPLIT, :],
                                isb[:, c0 * 8 : (c0 + cn) * 8],
                                cn * P,
                                cn * P,
                                H_OUT,
                                queue_num=(t * 2 + lohi) % nq,
                            )

                    acc = spp.tile([P, H_OUT], f32)
                    for b in range(NB):
                        s = spool.tile([P, P], bf16)
                        nc.vector.tensor_scalar(
                            s[:], iota[:], sw_sb[:, 0, b : b + 1],
                            sw_sb[:, 1, b : b + 1],
                            mybir.AluOpType.is_equal, mybir.AluOpType.mult,
                        )
                        nc.tensor.matmul(
                            acc[:], s[:], g[:, b, :],
                            start=(b == 0), stop=(b == NB - 1),
                        )
                    ax = xpool.tile([P, H_OUT], bf16)
                    nc.scalar.copy(ax[:], acc[:])
                    at_ps = tpp.tile([P, H_OUT], bf16)
                    for fc in range(KC):
                        nc.tensor.transpose(
                            at_ps[:, fc * P : (fc + 1) * P],
                            ax[:, fc * P : (fc + 1) * P],
                            ident[:],
                        )
                    at = tpool.tile([P, H_OUT], bf16)
                    nc.scalar.copy(at[:], at_ps[:])
                    oacc = opp.tile([P, H_OUT], f32)
                    for fc in range(KC):
                        nc.tensor.matmul(
                            oacc[:], at[:, fc * P : (fc + 1) * P], wsb[:, fc, :],
                            start=(fc == 0), stop=(fc == KC - 1),
                        )
                    o = opool.tile([P, H_OUT], f32)
                    nc.vector.tensor_scalar_max(o[:], oacc[:], 0.0)
                    # out goes on the ACT HWDGE queue: it waits on the tile's
                    # whole compute chain, and on the SP queue (strict FIFO
                    # per engine) it would block the next tiles' meta loads.
                    nc.scalar.dma_start(out[r0 : r0 + P, :], o[:])

    nc.compile()
    return nc


def _get_nc(NBL, NBH, reps=1):
    if (NBL, NBH, reps) not in _compiled:
        _compiled[(NBL, NBH, reps)] = _build_nc(NBL, NBH, reps)
    return _compiled[(NBL, NBH, reps)]


def _wrap_idx16(vals, n_groups, nb):
    """[n_groups, nb*128] linear gather indices -> [n_groups*128, nb*8] int16
    in the Q7 wrapped layout (idx i at [i%16, i//16], replicated to all 8
    groups of 16 partitions)."""
    wr = vals.reshape(n_groups, nb * 8, 16).transpose(0, 2, 1)  # [G, 16, nb*8]
    rep = np.tile(wr, (1, 8, 1))  # [G, 128, nb*8]
    return np.ascontiguousarray(rep.reshape(n_groups * P, nb * 8))


def _balance_tiles(deg_lo, deg_hi, cap=1024):
    """Assign each dst node to one of NTG tiles (<=128 nodes each), keeping
    per-tile lo/hi edge sums <= cap.  LPT greedy: largest nodes first, pick
    the feasible tile with the least total load."""
    n = deg_lo.shape[0]
    order = np.argsort(-(deg_lo + deg_hi), kind="stable")
    loads_lo = np.zeros(NTG, np.int64)
    loads_hi = np.zeros(NTG, np.int64)
    cnt = np.zeros(NTG, np.int64)
    tile_of = np.empty(n, np.int64)
    for node in order:
        dlo = deg_lo[node]
        dhi = deg_hi[node]
        feas = (cnt < P) & (loads_lo + dlo <= cap) & (loads_hi + dhi <= cap)
        if not feas.any():
            feas = cnt < P
        tot = np.where(feas, loads_lo + loads_hi, np.iinfo(np.int64).max)
        t = int(np.argmin(tot))
        tile_of[node] = t
        loads_lo[t] += dlo
        loads_hi[t] += dhi
        cnt[t] += 1
    return tile_of, loads_lo, loads_hi, cnt


def prepare(X, W, edge_src, edge_dst, edge_vals):
    """Host-side layout prep. Returns (nc, in_maps, perm)."""
    import ml_dtypes

    bf = ml_dtypes.bfloat16
    X = np.asarray(X, dtype=np.float32)
    W = np.ascontiguousarray(np.asarray(W, dtype=np.float32))
    src = np.asarray(edge_src).astype(np.int64)
    dst = np.asarray(edge_dst).astype(np.int64)
    ev = np.asarray(edge_vals, dtype=np.float32)
    E = src.shape[0]

    XR = np.zeros((XPAD, M_IN), dtype=bf)
    XR[:N_NODES] = X.astype(bf)
    Wb = W.astype(bf)

    half = (src >= SPLIT).astype(np.int64)   # 0 = lo table, 1 = hi table
    deg_lo = np.bincount(dst[half == 0], minlength=N_NODES)
    deg_hi = np.bincount(dst[half == 1], minlength=N_NODES)
    tile_of, loads_lo, loads_hi, cnt = _balance_tiles(deg_lo, deg_hi)

    NBL = max(1, int(np.ceil(loads_lo.max() / P)))
    NBH = max(1, int(np.ceil(loads_hi.max() / P)))
    NB = NBL + NBH

    # slot of each node within its tile; perm maps global out row -> node id
    order = np.argsort(tile_of, kind="stable")
    slot_of = np.empty(N_NODES, np.int64)
    starts = np.zeros(NTG + 1, np.int64)
    np.cumsum(np.bincount(tile_of, minlength=NTG), out=starts[1:])
    slot_of[order] = np.arange(N_NODES) - starts[tile_of[order]]
    perm = np.full(NTG * P, -1, np.int64)
    perm[tile_of * P + slot_of] = np.arange(N_NODES)

    # per-edge tile/slot; lay edges into blocks per (tile, half)
    et = tile_of[dst]
    eslot = slot_of[dst].astype(np.float32)
    key = et * 2 + half
    eorder = np.argsort(key, kind="stable")
    counts = np.bincount(key, minlength=NTG * 2)
    estarts = np.zeros(NTG * 2 + 1, np.int64)
    np.cumsum(counts, out=estarts[1:])
    skey = key[eorder]
    pos = np.arange(E, dtype=np.int64) - estarts[skey]
    shalf = half[eorder]
    sg = et[eorder]
    blk = pos // P + shalf * NBL             # block column in [0, NB)
    prt = pos % P
    row = sg * P + prt

    sw_arr = np.zeros((NTG * P, 2, NB), np.float32)
    sw_arr[row, 0, blk] = eslot[eorder]
    sw_arr[row, 1, blk] = ev[eorder]

    # linear per-(tile,half) gather index lists, padded with 0
    lin_lo = np.zeros((NTG, NBL * P), np.int16)
    lin_hi = np.zeros((NTG, NBH * P), np.int16)
    sidx = (src[eorder] - shalf * SPLIT).astype(np.int16)
    lo_m = shalf == 0
    lin_lo[sg[lo_m], pos[lo_m]] = sidx[lo_m]
    hi_m = ~lo_m
    lin_hi[sg[hi_m], pos[hi_m]] = sidx[hi_m]
    idx16_lo = _wrap_idx16(lin_lo, NTG, NBL)
    idx16_hi = _wrap_idx16(lin_hi, NTG, NBH)

    in_maps = [
        {
            "xr": XR,
            "w": Wb,
            "eidxlo": idx16_lo[c * ROWS_PC : (c + 1) * ROWS_PC],
            "eidxhi": idx16_hi[c * ROWS_PC : (c + 1) * ROWS_PC],
            "esw": sw_arr[c * ROWS_PC : (c + 1) * ROWS_PC],
        }
        for c in range(N_CORES)
    ]
    nc = _get_nc(NBL, NBH)
    return nc, in_maps, perm


def kernel(X, W, edge_src, edge_dst, edge_vals):
    from concourse.bass_utils import run_bass_kernel_spmd

    nc, in_maps, perm = prepare(X, W, edge_src, edge_dst, edge_vals)
    res = run_bass_kernel_spmd(nc, in_maps, core_ids=list(range(N_CORES)))
    outs = res.results
    all_rows = np.concatenate(
        [np.asarray(outs[c]["out"]) for c in range(N_CORES)], axis=0
    )
    full = np.empty((N_NODES, H_OUT), np.float32)
    valid = perm >= 0
    full[perm[valid]] = all_rows[valid]
    return full


# revision 10
# speedup vs baseline: 1.1810x; 1.1810x over previous
"""GCN layer kernel for Trainium2 (8 NeuronCores) — full computation on device.

Reference: Z = X @ W; agg = segment_sum(Z[edge_src] * edge_vals, edge_dst);
out = relu(agg).

Strategy: use linearity to reorder — relu((A_hat X) W) instead of
relu(A_hat (X W)).  Aggregating X FIRST means no core ever computes the full
50k-row projection (the baseline replicated a 26-GFLOP matmul on every core
and round-tripped a 51 MB Z through DRAM).  Each core owns 1/8 of the dst
nodes and:
  per 128-dst tile: Q7 dma_gather fetches the X rows of its source nodes
  (bf16, two tables split at row 25088 since gather indices are int16);
  a per-block selection matrix S[e,d] = (iota==slot)*w (built on DVE) folds
  the weighted segment-sum into PSUM-accumulated matmuls aggX[d,f] += S^T G;
  PE transposes aggX via identity-matmul; then a 4-step matmul with the
  replicated W [512,512] projects, and ReLU goes out.

Host does graph partitioning: dst nodes are packed 2D-greedily into
128-row tiles balancing lo/hi edge counts (so every tile needs exactly
NBL+NBH gather blocks with ~1% padding), plus the final unpermute.
"""

import numpy as np

N_NODES = 50000
M_IN = 512
H_OUT = 512
N_CORES = 8
P = 128
KC = M_IN // P                    # 4 contraction chunks for the projection
SPLIT = 25088                     # X table split (int16 gather index range)
XPAD = 2 * SPLIT                  # 50176 padded X rows
NT_PC = 50                        # dst tiles per core
NTG = N_CORES * NT_PC             # 400 global tiles
ROWS_PC = NT_PC * P               # 6400 out rows per core

_compiled = {}


def _build_nc(NBL, NBH, reps=1, nq=4):
    from contextlib import ExitStack
    from concourse import bacc, mybir
    from concourse import tile

    f32 = mybir.dt.float32
    bf16 = mybir.dt.bfloat16
    i16 = mybir.dt.int16
    NB = NBL + NBH

    # Each SWDGE queue_num runs descriptor generation on its own Q7 core
    # pair (dma_gather.cpp: cpu_id / 2 == queue_num), so rotating gathers
    # across 4 queues parallelizes the descgen that bounds this kernel.
    nc = bacc.Bacc(None, debug=False, num_swdge_queues=nq)

    xr = nc.declare_dram_parameter("xr", [XPAD, M_IN], bf16, isOutput=False)
    w = nc.declare_dram_parameter("w", [M_IN, H_OUT], bf16, isOutput=False)
    eidxlo = nc.declare_dram_parameter(
        "eidxlo", [ROWS_PC, NBL * 8], i16, isOutput=False
    )
    eidxhi = nc.declare_dram_parameter(
        "eidxhi", [ROWS_PC, NBH * 8], i16, isOutput=False
    )
    esw = nc.declare_dram_parameter("esw", [ROWS_PC, 2, NB], f32, isOutput=False)
    out = nc.declare_dram_parameter("out", [ROWS_PC, H_OUT], f32, isOutput=True)

    with tile.TileContext(nc) as tc:
        with ExitStack() as ctx:
            wpool = ctx.enter_context(tc.tile_pool(name="wpool", bufs=1))
            mpool = ctx.enter_context(tc.tile_pool(name="mpool", bufs=1))
            gpool = ctx.enter_context(tc.tile_pool(name="gpool", bufs=6))
            spool = ctx.enter_context(tc.tile_pool(name="spool", bufs=4))
            xpool = ctx.enter_context(tc.tile_pool(name="xpool", bufs=3))
            tpool = ctx.enter_context(tc.tile_pool(name="tpool", bufs=3))
            opool = ctx.enter_context(tc.tile_pool(name="opool", bufs=3))
            spp = ctx.enter_context(tc.tile_pool(name="spp", bufs=2, space="PSUM"))
            tpp = ctx.enter_context(tc.tile_pool(name="tpp", bufs=2, space="PSUM"))
            opp = ctx.enter_context(tc.tile_pool(name="opp", bufs=2, space="PSUM"))

            # reps>1 re-emits the whole computation (timing NEFFs only;
            # kernel() always uses reps=1)
            for _rep in range(reps):
                # ---- constants ----
                wsb = wpool.tile([P, KC, H_OUT], bf16, tag="wsb")
                for k in range(KC):
                    nc.sync.dma_start(wsb[:, k, :], w[k * P : (k + 1) * P, :])
                iota = wpool.tile([P, P], f32, tag="iota")
                nc.gpsimd.iota(
                    iota[:], pattern=[[1, P]], base=0, channel_multiplier=0,
                    allow_small_or_imprecise_dtypes=True,
                )
                piota = wpool.tile([P, 1], f32, tag="piota")
                nc.gpsimd.iota(
                    piota[:], pattern=[[0, 1]], base=0, channel_multiplier=1,
                    allow_small_or_imprecise_dtypes=True,
                )
                ident = wpool.tile([P, P], bf16, tag="ident")
                nc.vector.tensor_scalar(
                    ident[:], iota[:], piota[:, 0:1], None,
                    mybir.AluOpType.is_equal,
                )

                # ---- prefetch ALL per-tile metadata up front: keeps the SP
                # queue free of per-tile loads, so a tile's out-DMA (which
                # waits on its compute chain) can never block later tiles'
                # gather inputs through the SP engine's FIFO.
                mlo = mpool.tile([P, NT_PC, NBL * 8], i16, tag="mlo")
                mhi = mpool.tile([P, NT_PC, NBH * 8], i16, tag="mhi")
                msw = mpool.tile([P, NT_PC, 2, NB], f32, tag="msw")
                for t in range(NT_PC):
                    r0 = t * P
                    nc.sync.dma_start(mlo[:, t, :], eidxlo[r0 : r0 + P, :])
                    nc.sync.dma_start(mhi[:, t, :], eidxhi[r0 : r0 + P, :])
                    nc.sync.dma_start(msw[:, t, :, :], esw[r0 : r0 + P, :, :])

                # ---- per dst tile: gather + select-matmul + transpose+proj ----
                for t in range(NT_PC):
                    r0 = t * P
                    idxlo_sb = mlo[:, t, :]
                    idxhi_sb = mhi[:, t, :]
                    sw_sb = msw[:, t, :, :]

                    g = gpool.tile([P, NB, H_OUT], bf16)
                    # dma_gather breaks on HW above 1024 indices per call (the
                    # 1024-descriptor SWDGE ring) -> chunk into <=8-block calls
                    CH = 8
                    for nb, lohi, isb, off in (
                        (NBL, 0, idxlo_sb, 0),
                        (NBH, 1, idxhi_sb, NBL),
                    ):
                        for c0 in range(0, nb, CH):
                            cn = min(CH, nb - c0)
                            nc.gpsimd.dma_gather(
                                g[:, off + c0 : off + c0 + cn, :],
                                xr[lohi * SPLIT : (lohi + 1) * SPLIT, :],
                                isb[:, c0 * 8 : (c0 + cn) * 8],
                                cn * P,
                                cn * P,
                                H_OUT,
                                queue_num=(t * 2 + lohi) % nq,
                            )

                    acc = spp.tile([P, H_OUT], f32)
                    for b in range(NB):
                        s = spool.tile([P, P], bf16)
                        nc.vector.tensor_scalar(
                            s[:], iota[:], sw_sb[:, 0, b : b + 1],
                            sw_sb[:, 1, b : b + 1],
                            mybir.AluOpType.is_equal, mybir.AluOpType.mult,
                        )
                        nc.tensor.matmul(
                            acc[:], s[:], g[:, b, :],
                            start=(b == 0), stop=(b == NB - 1),
                        )
                    ax = xpool.tile([P, H_OUT], bf16)
                    nc.scalar.copy(ax[:], acc[:])
                    at_ps = tpp.tile([P, H_OUT], bf16)
                    for fc in range(KC):
                        nc.tensor.transpose(
                            at_ps[:, fc * P : (fc + 1) * P],
                            ax[:, fc * P : (fc + 1) * P],
                            ident[:],
                        )
                    at = tpool.tile([P, H_OUT], bf16)
                    nc.scalar.copy(at[:], at_ps[:])
                    oacc = opp.tile([P, H_OUT], f32)
                    for fc in range(KC):
                        nc.tensor.matmul(
                            oacc[:], at[:, fc * P : (fc + 1) * P], wsb[:, fc, :],
                            start=(fc == 0), stop=(fc == KC - 1),
                        )
                    o = opool.tile([P, H_OUT], f32)
                    nc.vector.tensor_scalar_max(o[:], oacc[:], 0.0)
                    nc.sync.dma_start(out[r0 : r0 + P, :], o[:])

    nc.compile()
    return nc


def _get_nc(NBL, NBH, reps=1):
    if (NBL, NBH, reps) not in _compiled:
        _compiled[(NBL, NBH, reps)] = _build_nc(NBL, NBH, reps)
    return _compiled[(NBL, NBH, reps)]


def _wrap_idx16(vals, n_groups, nb):
    """[n_groups, nb*128] linear gather indices -> [n_groups*128, nb*8] int16
    in the Q7 wrapped layout (idx i at [i%16, i//16], replicated to all 8
    groups of 16 partitions)."""
    wr = vals.reshape(n_groups, nb * 8, 16).transpose(0, 2, 1)  # [G, 16, nb*8]
    rep = np.tile(wr, (1, 8, 1))  # [G, 128, nb*8]
    return np.ascontiguousarray(rep.reshape(n_groups * P, nb * 8))


def _balance_tiles(deg_lo, deg_hi, cap=1024):
    """Assign each dst node to one of NTG tiles (<=128 nodes each), keeping
    per-tile lo/hi edge sums <= cap.  LPT greedy: largest nodes first, pick
    the feasible tile with the least total load."""
    n = deg_lo.shape[0]
    order = np.argsort(-(deg_lo + deg_hi), kind="stable")
    loads_lo = np.zeros(NTG, np.int64)
    loads_hi = np.zeros(NTG, np.int64)
    cnt = np.zeros(NTG, np.int64)
    tile_of = np.empty(n, np.int64)
    for node in order:
        dlo = deg_lo[node]
        dhi = deg_hi[node]
        feas = (cnt < P) & (loads_lo + dlo <= cap) & (loads_hi + dhi <= cap)
        if not feas.any():
            feas = cnt < P
        tot = np.where(feas, loads_lo + loads_hi, np.iinfo(np.int64).max)
        t = int(np.argmin(tot))
        tile_of[node] = t
        loads_lo[t] += dlo
        loads_hi[t] += dhi
        cnt[t] += 1
    return tile_of, loads_lo, loads_hi, cnt


def prepare(X, W, edge_src, edge_dst, edge_vals):
    """Host-side layout prep. Returns (nc, in_maps, perm)."""
    import ml_dtypes

    bf = ml_dtypes.bfloat16
    X = np.asarray(X, dtype=np.float32)
    W = np.ascontiguousarray(np.asarray(W, dtype=np.float32))
    src = np.asarray(edge_src).astype(np.int64)
    dst = np.asarray(edge_dst).astype(np.int64)
    ev = np.asarray(edge_vals, dtype=np.float32)
    E = src.shape[0]

    XR = np.zeros((XPAD, M_IN), dtype=bf)
    XR[:N_NODES] = X.astype(bf)
    Wb = W.astype(bf)

    half = (src >= SPLIT).astype(np.int64)   # 0 = lo table, 1 = hi table
    deg_lo = np.bincount(dst[half == 0], minlength=N_NODES)
    deg_hi = np.bincount(dst[half == 1], minlength=N_NODES)
    tile_of, loads_lo, loads_hi, cnt = _balance_tiles(deg_lo, deg_hi)

    NBL = max(1, int(np.ceil(loads_lo.max() / P)))
    NBH = max(1, int(np.ceil(loads_hi.max() / P)))
    NB = NBL + NBH

    # slot of each node within its tile; perm maps global out row -> node id
    order = np.argsort(tile_of, kind="stable")
    slot_of = np.empty(N_NODES, np.int64)
    starts = np.zeros(NTG + 1, np.int64)
    np.cumsum(np.bincount(tile_of, minlength=NTG), out=starts[1:])
    slot_of[order] = np.arange(N_NODES) - starts[tile_of[order]]
    perm = np.full(NTG * P, -1, np.int64)
    perm[tile_of * P + slot_of] = np.arange(N_NODES)

    # per-edge tile/slot; lay edges into blocks per (tile, half)
    et = tile_of[dst]
    eslot = slot_of[dst].astype(np.float32)
    key = et * 2 + half
    eorder = np.argsort(key, kind="stable")
    counts = np.bincount(key, minlength=NTG * 2)
    estarts = np.zeros(NTG * 2 + 1, np.int64)
    np.cumsum(counts, out=estarts[1:])
    skey = key[eorder]
    pos = np.arange(E, dtype=np.int64) - estarts[skey]
    shalf = half[eorder]
    sg = et[eorder]
    blk = pos // P + shalf * NBL             # block column in [0, NB)
    prt = pos % P
    row = sg * P + prt

    sw_arr = np.zeros((NTG * P, 2, NB), np.float32)
    sw_arr[row, 0, blk] = eslot[eorder]
    sw_arr[row, 1, blk] = ev[eorder]

    # linear per-(tile,half) gather index lists, padded with 0
    lin_lo = np.zeros((NTG, NBL * P), np.int16)
    lin_hi = np.zeros((NTG, NBH * P), np.int16)
    sidx = (src[eorder] - shalf * SPLIT).astype(np.int16)
    lo_m = shalf == 0
    lin_lo[sg[lo_m], pos[lo_m]] = sidx[lo_m]
    hi_m = ~lo_m
    lin_hi[sg[hi_m], pos[hi_m]] = sidx[hi_m]
    idx16_lo = _wrap_idx16(lin_lo, NTG, NBL)
    idx16_hi = _wrap_idx16(lin_hi, NTG, NBH)

    in_maps = [
        {
            "xr": XR,
            "w": Wb,
            "eidxlo": idx16_lo[c * ROWS_PC : (c + 1) * ROWS_PC],
            "eidxhi": idx16_hi[c * ROWS_PC : (c + 1) * ROWS_PC],
            "esw": sw_arr[c * ROWS_PC : (c + 1) * ROWS_PC],
        }
        for c in range(N_CORES)
    ]
    nc = _get_nc(NBL, NBH)
    return nc, in_maps, perm


def kernel(X, W, edge_src, edge_dst, edge_vals):
    from concourse.bass_utils import run_bass_kernel_spmd

    nc, in_maps, perm = prepare(X, W, edge_src, edge_dst, edge_vals)
    res = run_bass_kernel_spmd(nc, in_maps, core_ids=list(range(N_CORES)))
    outs = res.results
    all_rows = np.concatenate(
        [np.asarray(outs[c]["out"]) for c in range(N_CORES)], axis=0
    )
    full = np.empty((N_NODES, H_OUT), np.float32)
    valid = perm >= 0
    full[perm[valid]] = all_rows[valid]
    return full


# revision 12
# speedup vs baseline: 1.5237x; 1.2902x over previous
"""GCN layer kernel for Trainium2 (8 NeuronCores) — full computation on device.

Reference: Z = X @ W; agg = segment_sum(Z[edge_src] * edge_vals, edge_dst);
out = relu(agg).

Strategy: use linearity to reorder — relu((A_hat X) W) instead of
relu(A_hat (X W)).  Aggregating X FIRST means no core ever computes the full
50k-row projection (the baseline replicated a 26-GFLOP matmul on every core
and round-tripped a 51 MB Z through DRAM).  Each core owns 1/8 of the dst
nodes and:
  per 128-dst tile: Q7 dma_gather fetches the X rows of its source nodes
  (bf16, two tables split at row 25088 since gather indices are int16);
  a per-block selection matrix S[e,d] = (iota==slot)*w (built on DVE) folds
  the weighted segment-sum into PSUM-accumulated matmuls aggX[d,f] += S^T G;
  PE transposes aggX via identity-matmul; then a 4-step matmul with the
  replicated W [512,512] projects, and ReLU goes out.

Host does graph partitioning: dst nodes are packed 2D-greedily into
128-row tiles balancing lo/hi edge counts (so every tile needs exactly
NBL+NBH gather blocks with ~1% padding), plus the final unpermute.
"""

import numpy as np

N_NODES = 50000
M_IN = 512
H_OUT = 512
N_CORES = 8
P = 128
KC = M_IN // P                    # 4 contraction chunks for the projection
SPLIT = 25088                     # X table split (int16 gather index range)
XPAD = 2 * SPLIT                  # 50176 padded X rows
NT_PC = 50                        # dst tiles per core
NTG = N_CORES * NT_PC             # 400 global tiles
ROWS_PC = NT_PC * P               # 6400 out rows per core

_compiled = {}


def _build_nc(NBL, NBH, reps=1, nq=4):
    from contextlib import ExitStack
    from concourse import bacc, mybir
    from concourse import tile

    f32 = mybir.dt.float32
    bf16 = mybir.dt.bfloat16
    i16 = mybir.dt.int16
    NB = NBL + NBH

    # Each SWDGE queue_num runs descriptor generation on its own Q7 core
    # pair (dma_gather.cpp: cpu_id / 2 == queue_num), so rotating gathers
    # across 4 queues parallelizes the descgen that bounds this kernel.
    nc = bacc.Bacc(None, debug=False, num_swdge_queues=nq)

    xr = nc.declare_dram_parameter("xr", [XPAD, M_IN], bf16, isOutput=False)
    w = nc.declare_dram_parameter("w", [M_IN, H_OUT], bf16, isOutput=False)
    eidxlo = nc.declare_dram_parameter(
        "eidxlo", [ROWS_PC, NBL * 8], i16, isOutput=False
    )
    eidxhi = nc.declare_dram_parameter(
        "eidxhi", [ROWS_PC, NBH * 8], i16, isOutput=False
    )
    esw = nc.declare_dram_parameter("esw", [ROWS_PC, 2, NB], f32, isOutput=False)
    out = nc.declare_dram_parameter("out", [ROWS_PC, H_OUT], f32, isOutput=True)

    with tile.TileContext(nc) as tc:
        with ExitStack() as ctx:
            wpool = ctx.enter_context(tc.tile_pool(name="wpool", bufs=1))
            mpool = ctx.enter_context(tc.tile_pool(name="mpool", bufs=1))
            gpool = ctx.enter_context(tc.tile_pool(name="gpool", bufs=6))
            spool = ctx.enter_context(tc.tile_pool(name="spool", bufs=4))
            xpool = ctx.enter_context(tc.tile_pool(name="xpool", bufs=3))
            tpool = ctx.enter_context(tc.tile_pool(name="tpool", bufs=3))
            opool = ctx.enter_context(tc.tile_pool(name="opool", bufs=3))
            spp = ctx.enter_context(tc.tile_pool(name="spp", bufs=2, space="PSUM"))
            tpp = ctx.enter_context(tc.tile_pool(name="tpp", bufs=2, space="PSUM"))
            opp = ctx.enter_context(tc.tile_pool(name="opp", bufs=2, space="PSUM"))

            # reps>1 re-emits the whole computation (timing NEFFs only;
            # kernel() always uses reps=1)
            for _rep in range(reps):
                # ---- constants ----
                wsb = wpool.tile([P, KC, H_OUT], bf16, tag="wsb")
                for k in range(KC):
                    nc.sync.dma_start(wsb[:, k, :], w[k * P : (k + 1) * P, :])
                iota = wpool.tile([P, P], f32, tag="iota")
                nc.gpsimd.iota(
                    iota[:], pattern=[[1, P]], base=0, channel_multiplier=0,
                    allow_small_or_imprecise_dtypes=True,
                )
                iota_nb = wpool.tile([P, NB, P], f32, tag="iota_nb")
                nc.gpsimd.iota(
                    iota_nb[:], pattern=[[0, NB], [1, P]], base=0,
                    channel_multiplier=0, allow_small_or_imprecise_dtypes=True,
                )
                piota = wpool.tile([P, 1], f32, tag="piota")
                nc.gpsimd.iota(
                    piota[:], pattern=[[0, 1]], base=0, channel_multiplier=1,
                    allow_small_or_imprecise_dtypes=True,
                )
                ident = wpool.tile([P, P], bf16, tag="ident")
                nc.vector.tensor_scalar(
                    ident[:], iota[:], piota[:, 0:1], None,
                    mybir.AluOpType.is_equal,
                )

                # ---- prefetch ALL per-tile metadata up front: keeps the SP
                # queue free of per-tile loads, so a tile's out-DMA (which
                # waits on its compute chain) can never block later tiles'
                # gather inputs through the SP engine's FIFO.
                mlo = mpool.tile([P, NT_PC, NBL * 8], i16, tag="mlo")
                mhi = mpool.tile([P, NT_PC, NBH * 8], i16, tag="mhi")
                msw = mpool.tile([P, NT_PC, 2, NB], f32, tag="msw")
                for t in range(NT_PC):
                    r0 = t * P
                    nc.sync.dma_start(mlo[:, t, :], eidxlo[r0 : r0 + P, :])
                    nc.sync.dma_start(mhi[:, t, :], eidxhi[r0 : r0 + P, :])
                    nc.sync.dma_start(msw[:, t, :, :], esw[r0 : r0 + P, :, :])

                # ---- per dst tile: gather + select-matmul + transpose+proj ----
                for t in range(NT_PC):
                    r0 = t * P
                    idxlo_sb = mlo[:, t, :]
                    idxhi_sb = mhi[:, t, :]
                    sw_sb = msw[:, t, :, :]

                    g = gpool.tile([P, NB, H_OUT], bf16)
                    # dma_gather breaks on HW above 1024 indices per call (the
                    # 1024-descriptor SWDGE ring) -> chunk into <=8-block calls
                    CH = 8
                    for nb, lohi, isb, off in (
                        (NBL, 0, idxlo_sb, 0),
                        (NBH, 1, idxhi_sb, NBL),
                    ):
                        for c0 in range(0, nb, CH):
                            cn = min(CH, nb - c0)
                            nc.gpsimd.dma_gather(
                                g[:, off + c0 : off + c0 + cn, :],
                                xr[lohi * SPLIT : (lohi + 1) * SPLIT, :],
                                isb[:, c0 * 8 : (c0 + cn) * 8],
                                cn * P,
                                cn * P,
                                H_OUT,
                                queue_num=(t * 2 + lohi) % nq,
                            )

                    # batched S-build: 2 DVE instructions per tile instead of
                    # 16 — each DVE op excludes Q7 SWDGE from their shared
                    # SBUF port, so fewer/larger ops keep the gather descgen
                    # (the kernel's bottleneck) running.
                    s_all = spool.tile([P, NB, P], bf16)
                    nc.vector.tensor_tensor(
                        s_all[:], iota_nb[:],
                        sw_sb[:, 0, :].broadcast_to([P, NB, P]),
                        mybir.AluOpType.is_equal,
                    )
                    nc.vector.tensor_tensor(
                        s_all[:], s_all[:],
                        sw_sb[:, 1, :].broadcast_to([P, NB, P]),
                        mybir.AluOpType.mult,
                    )
                    acc = spp.tile([P, H_OUT], f32)
                    for b in range(NB):
                        nc.tensor.matmul(
                            acc[:], s_all[:, b, :], g[:, b, :],
                            start=(b == 0), stop=(b == NB - 1),
                        )
                    ax = xpool.tile([P, H_OUT], bf16)
                    nc.scalar.copy(ax[:], acc[:])
                    at_ps = tpp.tile([P, H_OUT], bf16)
                    for fc in range(KC):
                        nc.tensor.transpose(
                            at_ps[:, fc * P : (fc + 1) * P],
                            ax[:, fc * P : (fc + 1) * P],
                            ident[:],
                        )
                    at = tpool.tile([P, H_OUT], bf16)
                    nc.scalar.copy(at[:], at_ps[:])
                    oacc = opp.tile([P, H_OUT], f32)
                    for fc in range(KC):
                        nc.tensor.matmul(
                            oacc[:], at[:, fc * P : (fc + 1) * P], wsb[:, fc, :],
                            start=(fc == 0), stop=(fc == KC - 1),
                        )
                    o = opool.tile([P, H_OUT], f32)
                    nc.vector.tensor_scalar_max(o[:], oacc[:], 0.0)
                    nc.sync.dma_start(out[r0 : r0 + P, :], o[:])

    nc.compile()
    return nc


def _get_nc(NBL, NBH, reps=1):
    if (NBL, NBH, reps) not in _compiled:
        _compiled[(NBL, NBH, reps)] = _build_nc(NBL, NBH, reps)
    return _compiled[(NBL, NBH, reps)]


def _wrap_idx16(vals, n_groups, nb):
    """[n_groups, nb*128] linear gather indices -> [n_groups*128, nb*8] int16
    in the Q7 wrapped layout (idx i at [i%16, i//16], replicated to all 8
    groups of 16 partitions)."""
    wr = vals.reshape(n_groups, nb * 8, 16).transpose(0, 2, 1)  # [G, 16, nb*8]
    rep = np.tile(wr, (1, 8, 1))  # [G, 128, nb*8]
    return np.ascontiguousarray(rep.reshape(n_groups * P, nb * 8))


def _balance_tiles(deg_lo, deg_hi, cap=1024):
    """Assign each dst node to one of NTG tiles (<=128 nodes each), keeping
    per-tile lo/hi edge sums <= cap.  LPT greedy: largest nodes first, pick
    the feasible tile with the least total load."""
    n = deg_lo.shape[0]
    order = np.argsort(-(deg_lo + deg_hi), kind="stable")
    loads_lo = np.zeros(NTG, np.int64)
    loads_hi = np.zeros(NTG, np.int64)
    cnt = np.zeros(NTG, np.int64)
    tile_of = np.empty(n, np.int64)
    for node in order:
        dlo = deg_lo[node]
        dhi = deg_hi[node]
        feas = (cnt < P) & (loads_lo + dlo <= cap) & (loads_hi + dhi <= cap)
        if not feas.any():
            feas = cnt < P
        tot = np.where(feas, loads_lo + loads_hi, np.iinfo(np.int64).max)
        t = int(np.argmin(tot))
        tile_of[node] = t
        loads_lo[t] += dlo
        loads_hi[t] += dhi
        cnt[t] += 1
    return tile_of, loads_lo, loads_hi, cnt


def prepare(X, W, edge_src, edge_dst, edge_vals):
    """Host-side layout prep. Returns (nc, in_maps, perm)."""
    import ml_dtypes

    bf = ml_dtypes.bfloat16
    X = np.asarray(X, dtype=np.float32)
    W = np.ascontiguousarray(np.asarray(W, dtype=np.float32))
    src = np.asarray(edge_src).astype(np.int64)
    dst = np.asarray(edge_dst).astype(np.int64)
    ev = np.asarray(edge_vals, dtype=np.float32)
    E = src.shape[0]

    XR = np.zeros((XPAD, M_IN), dtype=bf)
    XR[:N_NODES] = X.astype(bf)
    Wb = W.astype(bf)

    half = (src >= SPLIT).astype(np.int64)   # 0 = lo table, 1 = hi table
    deg_lo = np.bincount(dst[half == 0], minlength=N_NODES)
    deg_hi = np.bincount(dst[half == 1], minlength=N_NODES)
    tile_of, loads_lo, loads_hi, cnt = _balance_tiles(deg_lo, deg_hi)

    NBL = max(1, int(np.ceil(loads_lo.max() / P)))
    NBH = max(1, int(np.ceil(loads_hi.max() / P)))
    NB = NBL + NBH

    # slot of each node within its tile; perm maps global out row -> node id
    order = np.argsort(tile_of, kind="stable")
    slot_of = np.empty(N_NODES, np.int64)
    starts = np.zeros(NTG + 1, np.int64)
    np.cumsum(np.bincount(tile_of, minlength=NTG), out=starts[1:])
    slot_of[order] = np.arange(N_NODES) - starts[tile_of[order]]
    perm = np.full(NTG * P, -1, np.int64)
    perm[tile_of * P + slot_of] = np.arange(N_NODES)

    # per-edge tile/slot; lay edges into blocks per (tile, half)
    et = tile_of[dst]
    eslot = slot_of[dst].astype(np.float32)
    key = et * 2 + half
    eorder = np.argsort(key, kind="stable")
    counts = np.bincount(key, minlength=NTG * 2)
    estarts = np.zeros(NTG * 2 + 1, np.int64)
    np.cumsum(counts, out=estarts[1:])
    skey = key[eorder]
    pos = np.arange(E, dtype=np.int64) - estarts[skey]
    shalf = half[eorder]
    sg = et[eorder]
    blk = pos // P + shalf * NBL             # block column in [0, NB)
    prt = pos % P
    row = sg * P + prt

    sw_arr = np.zeros((NTG * P, 2, NB), np.float32)
    sw_arr[row, 0, blk] = eslot[eorder]
    sw_arr[row, 1, blk] = ev[eorder]

    # linear per-(tile,half) gather index lists, padded with 0
    lin_lo = np.zeros((NTG, NBL * P), np.int16)
    lin_hi = np.zeros((NTG, NBH * P), np.int16)
    sidx = (src[eorder] - shalf * SPLIT).astype(np.int16)
    lo_m = shalf == 0
    lin_lo[sg[lo_m], pos[lo_m]] = sidx[lo_m]
    hi_m = ~lo_m
    lin_hi[sg[hi_m], pos[hi_m]] = sidx[hi_m]
    idx16_lo = _wrap_idx16(lin_lo, NTG, NBL)
    idx16_hi = _wrap_idx16(lin_hi, NTG, NBH)

    in_maps = [
        {
            "xr": XR,
            "w": Wb,
            "eidxlo": idx16_lo[c * ROWS_PC : (c + 1) * ROWS_PC],
            "eidxhi": idx16_hi[c * ROWS_PC : (c + 1) * ROWS_PC],
            "esw": sw_arr[c * ROWS_PC : (c + 1) * ROWS_PC],
        }
        for c in range(N_CORES)
    ]
    nc = _get_nc(NBL, NBH)
    return nc, in_maps, perm


def kernel(X, W, edge_src, edge_dst, edge_vals):
    from concourse.bass_utils import run_bass_kernel_spmd

    nc, in_maps, perm = prepare(X, W, edge_src, edge_dst, edge_vals)
    res = run_bass_kernel_spmd(nc, in_maps, core_ids=list(range(N_CORES)))
    outs = res.results
    all_rows = np.concatenate(
        [np.asarray(outs[c]["out"]) for c in range(N_CORES)], axis=0
    )
    full = np.empty((N_NODES, H_OUT), np.float32)
    valid = perm >= 0
    full[perm[valid]] = all_rows[valid]
    return full


# revision 16
# speedup vs baseline: 1.6023x; 1.0516x over previous
"""GCN layer kernel for Trainium2 (8 NeuronCores) — full computation on device.

Reference: Z = X @ W; agg = segment_sum(Z[edge_src] * edge_vals, edge_dst);
out = relu(agg).

Strategy: use linearity to reorder — relu((A_hat X) W) instead of
relu(A_hat (X W)).  Aggregating X FIRST means no core ever computes the full
50k-row projection (the baseline replicated a 26-GFLOP matmul on every core
and round-tripped a 51 MB Z through DRAM).  Each core owns 1/8 of the dst
nodes and:
  per 128-dst tile: Q7 dma_gather fetches the X rows of its source nodes
  (bf16, two tables split at row 25088 since gather indices are int16);
  a per-block selection matrix S[e,d] = (iota==slot)*w (built on DVE) folds
  the weighted segment-sum into PSUM-accumulated matmuls aggX[d,f] += S^T G;
  PE transposes aggX via identity-matmul; then a 4-step matmul with the
  replicated W [512,512] projects, and ReLU goes out.

Host does graph partitioning: dst nodes are packed 2D-greedily into
128-row tiles balancing lo/hi edge counts (so every tile needs exactly
NBL+NBH gather blocks with ~1% padding), plus the final unpermute.
"""

import numpy as np

N_NODES = 50000
M_IN = 512
H_OUT = 512
N_CORES = 8
P = 128
KC = M_IN // P                    # 4 contraction chunks for the projection
SPLIT = 25088                     # X table split (int16 gather index range)
XPAD = 2 * SPLIT                  # 50176 padded X rows
NT_PC = 50                        # dst tiles per core
NTG = N_CORES * NT_PC             # 400 global tiles
ROWS_PC = NT_PC * P               # 6400 out rows per core

_compiled = {}


def _build_nc(NBL, NBH, reps=1, nq=4):
    from contextlib import ExitStack
    from concourse import bacc, mybir
    from concourse import tile

    f32 = mybir.dt.float32
    bf16 = mybir.dt.bfloat16
    i16 = mybir.dt.int16
    NB = NBL + NBH

    # Each SWDGE queue_num runs descriptor generation on its own Q7 core
    # pair (dma_gather.cpp: cpu_id / 2 == queue_num), so rotating gathers
    # across 4 queues parallelizes the descgen that bounds this kernel.
    nc = bacc.Bacc(None, debug=False, num_swdge_queues=nq)

    xr = nc.declare_dram_parameter("xr", [XPAD, M_IN], bf16, isOutput=False)
    w = nc.declare_dram_parameter("w", [M_IN, H_OUT], bf16, isOutput=False)
    eidxlo = nc.declare_dram_parameter(
        "eidxlo", [ROWS_PC, NBL * 8], i16, isOutput=False
    )
    eidxhi = nc.declare_dram_parameter(
        "eidxhi", [ROWS_PC, NBH * 8], i16, isOutput=False
    )
    esw = nc.declare_dram_parameter("esw", [ROWS_PC, 2, NB], f32, isOutput=False)
    out = nc.declare_dram_parameter("out", [ROWS_PC, H_OUT], f32, isOutput=True)

    with tile.TileContext(nc) as tc:
        with ExitStack() as ctx:
            wpool = ctx.enter_context(tc.tile_pool(name="wpool", bufs=1))
            mpool = ctx.enter_context(tc.tile_pool(name="mpool", bufs=1))
            gpool = ctx.enter_context(tc.tile_pool(name="gpool", bufs=6))
            spool = ctx.enter_context(tc.tile_pool(name="spool", bufs=4))
            xpool = ctx.enter_context(tc.tile_pool(name="xpool", bufs=3))
            tpool = ctx.enter_context(tc.tile_pool(name="tpool", bufs=3))
            opool = ctx.enter_context(tc.tile_pool(name="opool", bufs=3))
            spp = ctx.enter_context(tc.tile_pool(name="spp", bufs=2, space="PSUM"))
            tpp = ctx.enter_context(tc.tile_pool(name="tpp", bufs=2, space="PSUM"))
            opp = ctx.enter_context(tc.tile_pool(name="opp", bufs=2, space="PSUM"))

            # reps>1 re-emits the whole computation (timing NEFFs only;
            # kernel() always uses reps=1)
            for _rep in range(reps):
                # ---- constants ----
                wsb = wpool.tile([P, KC, H_OUT], bf16, tag="wsb")
                for k in range(KC):
                    nc.sync.dma_start(wsb[:, k, :], w[k * P : (k + 1) * P, :])
                iota = wpool.tile([P, P], f32, tag="iota")
                nc.gpsimd.iota(
                    iota[:], pattern=[[1, P]], base=0, channel_multiplier=0,
                    allow_small_or_imprecise_dtypes=True,
                )
                iota_nb = wpool.tile([P, 2 * NB, P], f32, tag="iota_nb")
                nc.gpsimd.iota(
                    iota_nb[:], pattern=[[0, 2 * NB], [1, P]], base=0,
                    channel_multiplier=0, allow_small_or_imprecise_dtypes=True,
                )
                piota = wpool.tile([P, 1], f32, tag="piota")
                nc.gpsimd.iota(
                    piota[:], pattern=[[0, 1]], base=0, channel_multiplier=1,
                    allow_small_or_imprecise_dtypes=True,
                )
                ident = wpool.tile([P, P], bf16, tag="ident")
                nc.vector.tensor_scalar(
                    ident[:], iota[:], piota[:, 0:1], None,
                    mybir.AluOpType.is_equal,
                )

                # ---- prefetch ALL per-tile metadata up front: keeps the SP
                # queue free of per-tile loads, so a tile's out-DMA (which
                # waits on its compute chain) can never block later tiles'
                # gather inputs through the SP engine's FIFO.
                mlo = mpool.tile([P, NT_PC, NBL * 8], i16, tag="mlo")
                mhi = mpool.tile([P, NT_PC, NBH * 8], i16, tag="mhi")
                msw = mpool.tile([P, NT_PC, 2, NB], f32, tag="msw")
                for t in range(NT_PC):
                    r0 = t * P
                    nc.sync.dma_start(mlo[:, t, :], eidxlo[r0 : r0 + P, :])
                    nc.sync.dma_start(mhi[:, t, :], eidxhi[r0 : r0 + P, :])
                    nc.sync.dma_start(msw[:, t, :, :], esw[r0 : r0 + P, :, :])

                # ---- per dst tile: gather + select-matmul + transpose+proj ----
                s_pair = None
                for t in range(NT_PC):
                    r0 = t * P
                    idxlo_sb = mlo[:, t, :]
                    idxhi_sb = mhi[:, t, :]
                    sw_sb = msw[:, t, :, :]

                    # S matrices for tiles (t, t+1) built in one pass: every
                    # DVE instruction excludes Q7 SWDGE from their shared
                    # SBUF port, so use as few DVE ops as possible.
                    if t % 2 == 0:
                        tn = min(2, NT_PC - t)
                        s_pair = spool.tile([P, 2, NB, P], bf16)
                        nc.vector.tensor_tensor(
                            s_pair[:, :tn, :, :],
                            iota_nb[:, : tn * NB, :],
                            msw[:, t : t + tn, 0, :].broadcast_to([P, tn, NB, P]),
                            mybir.AluOpType.is_equal,
                        )
                        nc.vector.tensor_tensor(
                            s_pair[:, :tn, :, :], s_pair[:, :tn, :, :],
                            msw[:, t : t + tn, 1, :].broadcast_to([P, tn, NB, P]),
                            mybir.AluOpType.mult,
                        )
                    s_all = s_pair[:, t % 2, :, :]

                    g = gpool.tile([P, NB, H_OUT], bf16)
                    # dma_gather breaks on HW above 1024 indices per call (the
                    # 1024-descriptor SWDGE ring) -> chunk into <=8-block calls
                    CH = 8
                    for nb, lohi, isb, off in (
                        (NBL, 0, idxlo_sb, 0),
                        (NBH, 1, idxhi_sb, NBL),
                    ):
                        for c0 in range(0, nb, CH):
                            cn = min(CH, nb - c0)
                            nc.gpsimd.dma_gather(
                                g[:, off + c0 : off + c0 + cn, :],
                                xr[lohi * SPLIT : (lohi + 1) * SPLIT, :],
                                isb[:, c0 * 8 : (c0 + cn) * 8],
                                cn * P,
                                cn * P,
                                H_OUT,
                                queue_num=(t * 2 + lohi) % nq,
                            )

                    acc = spp.tile([P, H_OUT], f32)
                    for b in range(NB):
                        nc.tensor.matmul(
                            acc[:], s_all[:, b, :], g[:, b, :],
                            start=(b == 0), stop=(b == NB - 1),
                        )
                    ax = xpool.tile([P, H_OUT], bf16)
                    nc.scalar.copy(ax[:], acc[:])
                    at_ps = tpp.tile([P, H_OUT], bf16)
                    for fc in range(KC):
                        nc.tensor.transpose(
                            at_ps[:, fc * P : (fc + 1) * P],
                            ax[:, fc * P : (fc + 1) * P],
                            ident[:],
                        )
                    at = tpool.tile([P, H_OUT], bf16)
                    nc.scalar.copy(at[:], at_ps[:])
                    oacc = opp.tile([P, H_OUT], f32)
                    for fc in range(KC):
                        nc.tensor.matmul(
                            oacc[:], at[:, fc * P : (fc + 1) * P], wsb[:, fc, :],
                            start=(fc == 0), stop=(fc == KC - 1),
                        )
                    o = opool.tile([P, H_OUT], f32)
                    # ReLU on ACT (own SBUF port) rather than DVE
                    nc.scalar.activation(
                        o[:], oacc[:], mybir.ActivationFunctionType.Relu
                    )
                    nc.sync.dma_start(out[r0 : r0 + P, :], o[:])

    nc.compile()
    return nc


def _get_nc(NBL, NBH, reps=1):
    if (NBL, NBH, reps) not in _compiled:
        _compiled[(NBL, NBH, reps)] = _build_nc(NBL, NBH, reps)
    return _compiled[(NBL, NBH, reps)]


def _wrap_idx16(vals, n_groups, nb):
    """[n_groups, nb*128] linear gather indices -> [n_groups*128, nb*8] int16
    in the Q7 wrapped layout (idx i at [i%16, i//16], replicated to all 8
    groups of 16 partitions)."""
    wr = vals.reshape(n_groups, nb * 8, 16).transpose(0, 2, 1)  # [G, 16, nb*8]
    rep = np.tile(wr, (1, 8, 1))  # [G, 128, nb*8]
    return np.ascontiguousarray(rep.reshape(n_groups * P, nb * 8))


def _balance_tiles(deg_lo, deg_hi, cap=1024):
    """Assign each dst node to one of NTG tiles (<=128 nodes each), keeping
    per-tile lo/hi edge sums <= cap.  LPT greedy: largest nodes first, pick
    the feasible tile with the least total load."""
    n = deg_lo.shape[0]
    order = np.argsort(-(deg_lo + deg_hi), kind="stable")
    loads_lo = np.zeros(NTG, np.int64)
    loads_hi = np.zeros(NTG, np.int64)
    cnt = np.zeros(NTG, np.int64)
    tile_of = np.empty(n, np.int64)
    for node in order:
        dlo = deg_lo[node]
        dhi = deg_hi[node]
        feas = (cnt < P) & (loads_lo + dlo <= cap) & (loads_hi + dhi <= cap)
        if not feas.any():
            feas = cnt < P
        tot = np.where(feas, loads_lo + loads_hi, np.iinfo(np.int64).max)
        t = int(np.argmin(tot))
        tile_of[node] = t
        loads_lo[t] += dlo
        loads_hi[t] += dhi
        cnt[t] += 1
    return tile_of, loads_lo, loads_hi, cnt


def prepare(X, W, edge_src, edge_dst, edge_vals):
    """Host-side layout prep. Returns (nc, in_maps, perm)."""
    import ml_dtypes

    bf = ml_dtypes.bfloat16
    X = np.asarray(X, dtype=np.float32)
    W = np.ascontiguousarray(np.asarray(W, dtype=np.float32))
    src = np.asarray(edge_src).astype(np.int64)
    dst = np.asarray(edge_dst).astype(np.int64)
    ev = np.asarray(edge_vals, dtype=np.float32)
    E = src.shape[0]

    XR = np.zeros((XPAD, M_IN), dtype=bf)
    XR[:N_NODES] = X.astype(bf)
    Wb = W.astype(bf)

    half = (src >= SPLIT).astype(np.int64)   # 0 = lo table, 1 = hi table
    deg_lo = np.bincount(dst[half == 0], minlength=N_NODES)
    deg_hi = np.bincount(dst[half == 1], minlength=N_NODES)
    tile_of, loads_lo, loads_hi, cnt = _balance_tiles(deg_lo, deg_hi)

    NBL = max(1, int(np.ceil(loads_lo.max() / P)))
    NBH = max(1, int(np.ceil(loads_hi.max() / P)))
    NB = NBL + NBH

    # slot of each node within its tile; perm maps global out row -> node id
    order = np.argsort(tile_of, kind="stable")
    slot_of = np.empty(N_NODES, np.int64)
    starts = np.zeros(NTG + 1, np.int64)
    np.cumsum(np.bincount(tile_of, minlength=NTG), out=starts[1:])
    slot_of[order] = np.arange(N_NODES) - starts[tile_of[order]]
    perm = np.full(NTG * P, -1, np.int64)
    perm[tile_of * P + slot_of] = np.arange(N_NODES)

    # per-edge tile/slot; lay edges into blocks per (tile, half)
    et = tile_of[dst]
    eslot = slot_of[dst].astype(np.float32)
    key = et * 2 + half
    eorder = np.argsort(key, kind="stable")
    counts = np.bincount(key, minlength=NTG * 2)
    estarts = np.zeros(NTG * 2 + 1, np.int64)
    np.cumsum(counts, out=estarts[1:])
    skey = key[eorder]
    pos = np.arange(E, dtype=np.int64) - estarts[skey]
    shalf = half[eorder]
    sg = et[eorder]
    blk = pos // P + shalf * NBL             # block column in [0, NB)
    prt = pos % P
    row = sg * P + prt

    sw_arr = np.zeros((NTG * P, 2, NB), np.float32)
    sw_arr[row, 0, blk] = eslot[eorder]
    sw_arr[row, 1, blk] = ev[eorder]

    # linear per-(tile,half) gather index lists, padded with 0
    lin_lo = np.zeros((NTG, NBL * P), np.int16)
    lin_hi = np.zeros((NTG, NBH * P), np.int16)
    sidx = (src[eorder] - shalf * SPLIT).astype(np.int16)
    lo_m = shalf == 0
    lin_lo[sg[lo_m], pos[lo_m]] = sidx[lo_m]
    hi_m = ~lo_m
    lin_hi[sg[hi_m], pos[hi_m]] = sidx[hi_m]
    idx16_lo = _wrap_idx16(lin_lo, NTG, NBL)
    idx16_hi = _wrap_idx16(lin_hi, NTG, NBH)

    in_maps = [
        {
            "xr": XR,
            "w": Wb,
            "eidxlo": idx16_lo[c * ROWS_PC : (c + 1) * ROWS_PC],
            "eidxhi": idx16_hi[c * ROWS_PC : (c + 1) * ROWS_PC],
            "esw": sw_arr[c * ROWS_PC : (c + 1) * ROWS_PC],
        }
        for c in range(N_CORES)
    ]
    nc = _get_nc(NBL, NBH)
    return nc, in_maps, perm


def kernel(X, W, edge_src, edge_dst, edge_vals):
    from concourse.bass_utils import run_bass_kernel_spmd

    nc, in_maps, perm = prepare(X, W, edge_src, edge_dst, edge_vals)
    res = run_bass_kernel_spmd(nc, in_maps, core_ids=list(range(N_CORES)))
    outs = res.results
    all_rows = np.concatenate(
        [np.asarray(outs[c]["out"]) for c in range(N_CORES)], axis=0
    )
    full = np.empty((N_NODES, H_OUT), np.float32)
    valid = perm >= 0
    full[perm[valid]] = all_rows[valid]
    return full
